# revision 22
# baseline (speedup 1.0000x reference)
"""DGL-style cross attention (GNN message passing) on 8 Trainium2 NeuronCores.

Sharding: nodes (and their q rows / output rows) are partitioned across the 8
cores; edges are partitioned by dst-node owner so the softmax-style segment-sum
normalization is core-local.  The k/v "halo" is handled by replicating a fused
bf16 KV table ([N, 512] = k row ++ v row) in every core's DRAM (recomputed
locally from the full input - cheaper than an all-gather at ~62 GB/s), and
per-edge rows are fetched with gpsimd dma_gather (SWDGE Ant gather).

Nodes are assigned to (core, block, lane) with a greedy in-degree balancer so
every 128-node dst block has a near-equal edge count - the SPMD program is
identical on all 8 cores, so padding waste is set by the LARGEST block.

Per dst block of 128 nodes the edge pipeline is:
  dma_gather kv[src] (two calls - int16 indices only reach 32767, so the
  table is gathered as two halves), dma_gather q[dst]
  score = exp(clip(rowdot(k, q))/sqrt(dk))          (DVE mult+reduce, ACT exp)
  segment sum of [score*v | score] via an indicator matmul into PSUM
  out_block = (wv / z) @ Wo.T + bo                  (PE transpose + matmul)
"""

import sys

for _p in ("/opt/trn_rl_repo",):
    if _p not in sys.path:
        sys.path.append(_p)

import heapq
import numpy as np
from contextlib import ExitStack

from concourse import bass, bacc, mybir, tile, library_config
from concourse.bass_utils import run_bass_kernel_spmd
from concourse.masks import make_identity

F32 = mybir.dt.float32
F32R = mybir.dt.float32r
BF16 = mybir.dt.bfloat16
I16 = mybir.dt.int16
AX = mybir.AxisListType
OP = mybir.AluOpType
ACTF = mybir.ActivationFunctionType

P = 128
HID = 256
HEADS = 8
DK = 32
SCALE = float(np.sqrt(DK))
CLIP = 10.0
CLIP_RAW = CLIP * SCALE  # clip before dividing by SCALE (equivalent)

N_CORES = 8

# dtype knobs ---------------------------------------------------------------
TABLE_DT = BF16   # dtype of kv_tab / q_tab in DRAM + gathered tiles
XF_DT = BF16      # dtype of the replicated x^T used for the kv projection
SEG_DT = BF16     # dtype of the segment-sum matmul operands (mask + wv)


def _cdiv(a, b):
    return -(-a // b)


def _np_dt(dt):
    return mybir.dt.np(dt)


class _Plan:
    """Host-side graph partition with load-balanced dst blocks."""

    def __init__(self, n_nodes, src, dst):
        self.n_nodes = n_nodes
        nblk_total = _cdiv(n_nodes, P)
        nblk_total = _cdiv(nblk_total, N_CORES) * N_CORES
        self.nblk = nblk_total // N_CORES          # blocks per core
        self.npad = self.nblk * P                  # node slots per core
        self.nkv = _cdiv(n_nodes, P) * P           # padded kv table rows
        self.split = (self.nkv // 2 // P) * P      # kv table half boundary

        deg = np.bincount(dst, minlength=n_nodes)
        # greedy balanced assignment: heaviest nodes first onto lightest block
        order = np.argsort(-deg, kind="stable")
        heap = [(0, b, 0) for b in range(nblk_total)]  # (load, block, n_nodes)
        heapq.heapify(heap)
        node_block = np.empty(n_nodes, np.int32)
        node_lane = np.empty(n_nodes, np.int32)
        for nid in order:
            load, b, cnt = heapq.heappop(heap)
            node_block[nid] = b
            node_lane[nid] = cnt
            cnt += 1
            if cnt < P:
                heapq.heappush(heap, (load + int(deg[nid]), b, cnt))
        self.node_block = node_block
        self.node_lane = node_lane
        # slot id within owner core: [0, npad)
        self.node_core = node_block // self.nblk
        self.node_slot = (node_block % self.nblk) * P + node_lane

        # per-(core,block,group) edge counts -> global S0/S1
        e_core = self.node_core[dst]
        e_blk = node_block[dst].astype(np.int64)
        e_grp = (src >= self.split).astype(np.int64)
        cnt = np.bincount(e_blk * 2 + e_grp, minlength=nblk_total * 2)
        cnt = cnt.reshape(nblk_total, 2)
        self.s0 = int(_cdiv(int(cnt[:, 0].max()), P))
        self.s1 = int(_cdiv(int(cnt[:, 1].max()), P))
        self.st = self.s0 + self.s1

        S0, S1, ST = self.s0, self.s1, self.st
        NBLK = self.nblk
        self.core_arrays = []
        for m in range(N_CORES):
            sel = e_core == m
            s_m = src[sel].astype(np.int64)
            blk = (e_blk[sel] % NBLK).astype(np.int64)
            dslot = self.node_slot[dst[sel]].astype(np.int64)
            grp = (s_m >= self.split).astype(np.int64)
            key = blk * 2 + grp
            order = np.argsort(key, kind="stable")
            s_m, blk, dslot, grp, key = (a[order] for a in
                                         (s_m, blk, dslot, grp, key))
            seg_cnt = np.bincount(key, minlength=NBLK * 2)
            start = np.zeros(NBLK * 2, np.int64)
            start[1:] = np.cumsum(seg_cnt)[:-1]
            j = np.arange(len(s_m)) - start[key]        # rank within segment
            i_blk = j + grp * (S0 * P)                  # slot id within block

            kv0 = np.zeros((NBLK, S0 * P), np.int64)
            kv1 = np.zeros((NBLK, S1 * P), np.int64)
            qi = np.zeros((NBLK, ST * P), np.int64)
            dstl = np.full((NBLK, ST * P), 999.0, np.float32)
            g0 = grp == 0
            kv0[blk[g0], j[g0]] = s_m[g0]
            g1 = grp == 1
            kv1[blk[g1], j[g1]] = s_m[g1] - self.split
            qi[blk, i_blk] = dslot
            dstl[blk, i_blk] = (dslot % P).astype(np.float32)

            self.core_arrays.append({
                "kvi0": self._wrap16(kv0),
                "kvi1": self._wrap16(kv1),
                "qi": self._wrap16(qi),
                "dstl": self._slotf(dstl),
            })

    @staticmethod
    def _wrap16(x):
        """[NBLK, n] flat slot-order indices -> [128, NBLK*(n//16)] int16
        (index i at [i % 16, i // 16], replicated for the 8 Q7 cores)."""
        nblk, n = x.shape
        w = x.reshape(nblk, n // 16, 16).transpose(0, 2, 1)   # [NBLK, 16, n/16]
        w = np.tile(w, (1, 8, 1))                             # [NBLK, 128, n/16]
        w = w.transpose(1, 0, 2).reshape(P, nblk * (n // 16))
        return np.ascontiguousarray(w.astype(np.int16))

    @staticmethod
    def _slotf(x):
        """[NBLK, n] slot-order floats -> [128, NBLK*(n//128)] (p = slot%128)."""
        nblk, n = x.shape
        w = x.reshape(nblk, n // P, P).transpose(2, 0, 1)
        return np.ascontiguousarray(w.reshape(P, nblk * (n // P)))


def _build_program(plan):
    S0, S1, ST = plan.s0, plan.s1, plan.st
    NBLK = plan.nblk
    NPAD = plan.npad
    NKV = plan.nkv
    SPLIT = plan.split

    nc = bacc.Bacc("TRN2", target_bir_lowering=False, debug=False,
                   num_devices=N_CORES)

    def inp(name, shape, dt):
        return nc.dram_tensor(name, shape, dt, kind="ExternalInput").ap()

    xT_own = inp("xT_own", [2, P, NPAD], F32R)
    xT_full = inp("xT_full", [2, P, NKV], XF_DT)
    wqT = inp("wqT", [2, P, HID], F32R)
    wkvT = inp("wkvT", [2, P, 2 * HID], XF_DT)
    woT = inp("woT", [2, P, HID], F32)
    bq_rep = inp("bq_rep", [P, HID], F32)
    bo_rep = inp("bo_rep", [P, HID], F32)
    kvi0_in = inp("kvi0", [P, NBLK * S0 * 8], I16)
    kvi1_in = inp("kvi1", [P, NBLK * S1 * 8], I16)
    qi_in = inp("qi", [P, NBLK * ST * 8], I16)
    dstl_in = inp("dstl", [P, NBLK * ST], F32)
    iota_in = inp("iota_row", [P, P], F32)

    out_ap = nc.dram_tensor("out", [NPAD, HID], F32, kind="ExternalOutput").ap()

    with tile.TileContext(nc) as tc, ExitStack() as ctx:
        dram = ctx.enter_context(tc.tile_pool(name="dram", bufs=1, space="DRAM"))
        q_tab = dram.tile([NPAD, HID], TABLE_DT)
        kv_tab = dram.tile([NKV, 2 * HID], TABLE_DT)

        const = ctx.enter_context(tc.tile_pool(name="const", bufs=1))
        nc.gpsimd.load_library(library_config.mlp)
        ident = const.tile([P, P], F32)
        make_identity(nc, ident[:])
        iota_sb = const.tile([P, P], F32)
        nc.sync.dma_start(out=iota_sb[:], in_=iota_in[:])
        bq_sb = const.tile([P, HID], F32)
        nc.sync.dma_start(out=bq_sb[:], in_=bq_rep[:])
        bo_sb = const.tile([P, HID], F32)
        nc.sync.dma_start(out=bo_sb[:], in_=bo_rep[:])
        wq_sb = const.tile([P, 2, HID], F32R)
        wkv_sb = const.tile([P, 2, 2 * HID], XF_DT)
        wo_sb = const.tile([P, 2, HID], F32)
        for c in range(2):
            nc.sync.dma_start(out=wq_sb[:, c, :], in_=wqT[c])
            nc.sync.dma_start(out=wkv_sb[:, c, :], in_=wkvT[c])
            nc.sync.dma_start(out=wo_sb[:, c, :], in_=woT[c])
        kvi0_sb = const.tile([P, NBLK * S0 * 8], I16)
        nc.sync.dma_start(out=kvi0_sb[:], in_=kvi0_in[:])
        kvi1_sb = const.tile([P, NBLK * S1 * 8], I16)
        nc.sync.dma_start(out=kvi1_sb[:], in_=kvi1_in[:])
        qi_sb = const.tile([P, NBLK * ST * 8], I16)
        nc.sync.dma_start(out=qi_sb[:], in_=qi_in[:])
        dstl_sb = const.tile([P, NBLK * ST], F32)
        nc.sync.dma_start(out=dstl_sb[:], in_=dstl_in[:])

        # ---------------- phase 1: projections -> q_tab, kv_tab ------------
        with ExitStack() as p1:
            own_pool = p1.enter_context(tc.tile_pool(name="own", bufs=1))
            slab_pool = p1.enter_context(tc.tile_pool(name="slab", bufs=2))
            qs_pool = p1.enter_context(tc.tile_pool(name="qs", bufs=3))
            kvs_pool = p1.enter_context(tc.tile_pool(name="kvs", bufs=4))
            psq = p1.enter_context(tc.tile_pool(name="psq", bufs=2, space="PSUM"))
            pskv = p1.enter_context(tc.tile_pool(name="pskv", bufs=3, space="PSUM"))

            xo_sb = own_pool.tile([P, 2, NPAD], F32R)
            for c in range(2):
                nc.sync.dma_start(out=xo_sb[:, c, :], in_=xT_own[c])

            for b in range(NBLK):
                ps = psq.tile([P, HID], F32, space="PSUM")
                for c in range(2):
                    nc.tensor.matmul(
                        out=ps[:],
                        lhsT=xo_sb[:, c, b * P:(b + 1) * P],
                        rhs=wq_sb[:, c, :],
                        start=(c == 0), stop=(c == 1))
                qs = qs_pool.tile([P, HID], TABLE_DT)
                nc.vector.tensor_tensor(qs[:], ps[:], bq_sb[:], op=OP.add)
                nc.sync.dma_start(out=q_tab[b * P:(b + 1) * P, :], in_=qs[:])

            SLAB = 2048
            nslab = _cdiv(NKV, SLAB)
            for s in range(nslab):
                w = min(SLAB, NKV - s * SLAB)
                xs = slab_pool.tile([P, 2, SLAB], XF_DT)
                for c in range(2):
                    nc.sync.dma_start(out=xs[:, c, :w],
                                      in_=xT_full[c, :, s * SLAB:s * SLAB + w])
                for k in range(w // P):
                    ps = pskv.tile([P, 2 * HID], F32, space="PSUM")
                    for c in range(2):
                        nc.tensor.matmul(out=ps[:],
                                         lhsT=xs[:, c, k * P:(k + 1) * P],
                                         rhs=wkv_sb[:, c, :],
                                         start=(c == 0), stop=(c == 1))
                    kvs = kvs_pool.tile([P, 2 * HID], TABLE_DT)
                    row = s * SLAB // P + k
                    if row % 2 == 0:
                        nc.vector.tensor_copy(kvs[:], ps[:])
                    else:
                        nc.scalar.copy(kvs[:], ps[:])
                    nc.sync.dma_start(out=kv_tab[row * P:(row + 1) * P, :],
                                      in_=kvs[:])

        # ---------------- phase 2+3: edge pipeline per dst block -----------
        kv_pool = ctx.enter_context(tc.tile_pool(name="kvt", bufs=2))
        qd_pool = ctx.enter_context(tc.tile_pool(name="qdt", bufs=2))
        me_pool = ctx.enter_context(tc.tile_pool(name="me", bufs=2))
        prod_pool = ctx.enter_context(tc.tile_pool(name="prod", bufs=2))
        work_pool = ctx.enter_context(tc.tile_pool(name="work", bufs=2))
        sc_pool = ctx.enter_context(tc.tile_pool(name="sc", bufs=2))
        se_pool = ctx.enter_context(tc.tile_pool(name="se", bufs=2))
        rz_pool = ctx.enter_context(tc.tile_pool(name="rz", bufs=2))
        op_pool = ctx.enter_context(tc.tile_pool(name="opre", bufs=2))
        ots_pool = ctx.enter_context(tc.tile_pool(name="ots", bufs=2))
        outs_pool = ctx.enter_context(tc.tile_pool(name="outs", bufs=3))
        acc_ps = ctx.enter_context(tc.tile_pool(name="acc", bufs=2, space="PSUM"))
        ot_psp = ctx.enter_context(tc.tile_pool(name="otp", bufs=2, space="PSUM"))
        out_psp = ctx.enter_context(tc.tile_pool(name="outp", bufs=2, space="PSUM"))

        MAXSUB = 8  # dma_gather handles at most 1024 indices per call

        def gather_chunks(out_tile, t_lo, n_sub, in_ap, idx_sb, col_base, elem):
            off = 0
            while off < n_sub:
                c = min(MAXSUB, n_sub - off)
                nc.gpsimd.dma_gather(
                    out_ap=out_tile[:, t_lo + off:t_lo + off + c, :],
                    in_ap=in_ap,
                    idxs_ap=idx_sb[:, col_base + off * 8:col_base + (off + c) * 8],
                    num_idxs=c * P, num_idxs_reg=c * P, elem_size=elem)
                off += c

        for b in range(NBLK):
            kvt = kv_pool.tile([P, ST, 2 * HID], TABLE_DT)
            gather_chunks(kvt, 0, S0, kv_tab[0:SPLIT, :], kvi0_sb,
                          b * S0 * 8, 2 * HID)
            gather_chunks(kvt, S0, S1, kv_tab[SPLIT:, :], kvi1_sb,
                          b * S1 * 8, 2 * HID)
            qdt = qd_pool.tile([P, ST, HID], TABLE_DT)
            gather_chunks(qdt, 0, ST, q_tab[:, :], qi_sb,
                          b * ST * 8, HID)

            me = me_pool.tile([P, ST, P], SEG_DT)
            nc.vector.tensor_tensor(
                me[:],
                dstl_sb[:, b * ST:(b + 1) * ST].unsqueeze(2)
                .broadcast_to([P, ST, P]),
                iota_sb[:].unsqueeze(1).broadcast_to([P, ST, P]),
                op=OP.is_equal)

            prod = prod_pool.tile([P, ST, HID], F32)
            nc.vector.tensor_tensor(prod[:], kvt[:, :, 0:HID],
                                    qdt[:], op=OP.mult)
            sc = sc_pool.tile([P, ST, HEADS], F32)
            nc.vector.tensor_reduce(
                sc[:],
                prod[:].rearrange("p s (h d) -> p s h d", h=HEADS),
                axis=AX.X, op=OP.add)
            nc.vector.tensor_scalar(sc[:], sc[:], CLIP_RAW, -CLIP_RAW,
                                    op0=OP.min, op1=OP.max)
            se = se_pool.tile([P, ST, HEADS], F32)
            nc.scalar.activation(se[:], sc[:], func=ACTF.Exp, scale=1.0 / SCALE)

            work = work_pool.tile([P, ST, HID + HEADS], SEG_DT)
            nc.vector.tensor_tensor(
                work[:, :, 0:HID].rearrange("p s (h d) -> p s h d", h=HEADS),
                kvt[:, :, HID:2 * HID].rearrange("p s (h d) -> p s h d", h=HEADS),
                se[:].unsqueeze(3).broadcast_to([P, ST, HEADS, DK]),
                op=OP.mult)
            nc.vector.tensor_copy(work[:, :, HID:HID + HEADS], se[:])

            acc = acc_ps.tile([P, HID + HEADS], F32, space="PSUM")
            for t in range(ST):
                nc.tensor.matmul(out=acc[:],
                                 lhsT=me[:, t, :],
                                 rhs=work[:, t, 0:HID + HEADS],
                                 start=(t == 0), stop=(t == ST - 1))

            # normalize + output projection
            nc.vector.tensor_scalar_add(acc[:, HID:HID + HEADS],
                                        acc[:, HID:HID + HEADS], 1e-30)
            rz = rz_pool.tile([P, HEADS], F32)
            nc.vector.reciprocal(rz[:], acc[:, HID:HID + HEADS])
            op_sb = op_pool.tile([P, HID], F32)
            nc.vector.tensor_tensor(
                op_sb[:].rearrange("p (h d) -> p h d", h=HEADS),
                acc[:, 0:HID].rearrange("p (h d) -> p h d", h=HEADS),
                rz[:].unsqueeze(2).broadcast_to([P, HEADS, DK]),
                op=OP.mult)
            ot_ps = ot_psp.tile([P, 2, P], F32, space="PSUM")
            for c in range(2):
                nc.tensor.transpose(ot_ps[:, c, :], op_sb[:, c * P:(c + 1) * P],
                                    ident[:])
            ot_sb = ots_pool.tile([P, 2, P], F32)
            nc.scalar.copy(ot_sb[:], ot_ps[:])
            out_ps = out_psp.tile([P, HID], F32, space="PSUM")
            for c in range(2):
                nc.tensor.matmul(out=out_ps[:],
                                 lhsT=ot_sb[:, c, :],
                                 rhs=wo_sb[:, c, :],
                                 start=(c == 0), stop=(c == 1))
            out_sb = outs_pool.tile([P, HID], F32)
            nc.vector.tensor_tensor(out_sb[:], out_ps[:], bo_sb[:], op=OP.add)
            nc.sync.dma_start(out=out_ap[b * P:(b + 1) * P, :], in_=out_sb[:])

    nc.compile()
    return nc


_PROG_CACHE = {}


def _get_program(plan):
    key = (plan.n_nodes, plan.s0, plan.s1)
    if key not in _PROG_CACHE:
        _PROG_CACHE[key] = _build_program(plan)
    return _PROG_CACHE[key]


def prepare(inputs, Wq, bq, Wk, Wv, Wo, bo, src, dst):
    inputs = np.asarray(inputs, np.float32)
    Wq = np.asarray(Wq, np.float32)
    bq = np.asarray(bq, np.float32)
    Wk = np.asarray(Wk, np.float32)
    Wv = np.asarray(Wv, np.float32)
    Wo = np.asarray(Wo, np.float32)
    bo = np.asarray(bo, np.float32)
    src = np.asarray(src, np.int64)
    dst = np.asarray(dst, np.int64)

    n, hid = inputs.shape
    assert hid == HID
    plan = _Plan(n, src, dst)
    nc = _get_program(plan)

    xT_full = np.zeros((2, P, plan.nkv), np.float32)
    xT_full[0, :, :n] = inputs.T[0:P, :]
    xT_full[1, :, :n] = inputs.T[P:2 * P, :]
    xT_full = xT_full.astype(_np_dt(XF_DT))
    wqT = np.ascontiguousarray(Wq.T.reshape(2, P, HID))
    wkvT = np.concatenate([Wk.T, Wv.T], axis=1).reshape(2, P, 2 * HID)
    wkvT = np.ascontiguousarray(wkvT).astype(_np_dt(XF_DT))
    woT = np.ascontiguousarray(Wo.T.reshape(2, P, HID))
    bq_rep = np.ascontiguousarray(np.broadcast_to(bq, (P, HID)))
    bo_rep = np.ascontiguousarray(np.broadcast_to(bo, (P, HID)))
    iota_row = np.ascontiguousarray(
        np.broadcast_to(np.arange(P, dtype=np.float32), (P, P)))

    # per-core x_own in (block, lane) slot order
    in_maps = []
    for m in range(N_CORES):
        sel = plan.node_core == m
        nids = np.nonzero(sel)[0]
        slots = plan.node_slot[nids]
        xo_rows = np.zeros((plan.npad, HID), np.float32)
        xo_rows[slots] = inputs[nids]
        xo = np.ascontiguousarray(
            xo_rows.T.reshape(2, P, plan.npad))
        ca = plan.core_arrays[m]
        in_maps.append({
            "xT_own": xo,
            "xT_full": xT_full,
            "wqT": wqT,
            "wkvT": wkvT,
            "woT": woT,
            "bq_rep": bq_rep,
            "bo_rep": bo_rep,
            "kvi0": ca["kvi0"],
            "kvi1": ca["kvi1"],
            "qi": ca["qi"],
            "dstl": ca["dstl"],
            "iota_row": iota_row,
        })
    return plan, nc, in_maps


def assemble(plan, res):
    n = plan.n_nodes
    out = np.zeros((n, HID), np.float32)
    for m in range(N_CORES):
        sel = plan.node_core == m
        nids = np.nonzero(sel)[0]
        slots = plan.node_slot[nids]
        out[nids] = np.asarray(res.results[m]["out"], np.float32)[slots]
    return out


def kernel(**inputs):
    plan, nc, in_maps = prepare(**inputs)
    res = run_bass_kernel_spmd(nc, in_maps, core_ids=list(range(N_CORES)))
    return assemble(plan, res)


# revision 31
# speedup vs baseline: 1.5485x; 1.5485x over previous
"""DGL-style cross attention (GNN message passing) on 8 Trainium2 NeuronCores.

Sharding: nodes (and their q rows / output rows) are partitioned across the 8
cores; edges are partitioned by dst-node owner so the softmax-style segment-sum
normalization is core-local.  The k/v "halo" is handled by replicating a fused
bf16 KV table ([N, 512] = k row ++ v row) in every core's DRAM (recomputed
locally from the full input - cheaper than an all-gather at ~62 GB/s), and
per-edge rows are fetched with gpsimd dma_gather (SWDGE Ant gather).

Nodes are assigned to (core, block, lane) with a greedy in-degree balancer so
every 128-node dst block has a near-equal edge count - the SPMD program is
identical on all 8 cores, so padding waste is set by the LARGEST block.

Per dst block of 128 nodes the edge pipeline is:
  dma_gather kv[src] (two calls - int16 indices only reach 32767, so the
  table is gathered as two halves), dma_gather q[dst]
  score = exp(clip(rowdot(k, q))/sqrt(dk))          (DVE mult+reduce, ACT exp)
  segment sum of [score*v | score] via an indicator matmul into PSUM
  out_block = (wv / z) @ Wo.T + bo                  (PE transpose + matmul)
"""

import sys

for _p in ("/opt/trn_rl_repo",):
    if _p not in sys.path:
        sys.path.append(_p)

import heapq
import numpy as np
from contextlib import ExitStack

from concourse import bass, bacc, mybir, tile, library_config
from concourse.bass_utils import run_bass_kernel_spmd
from concourse.masks import make_identity

F32 = mybir.dt.float32
F32R = mybir.dt.float32r
BF16 = mybir.dt.bfloat16
I16 = mybir.dt.int16
AX = mybir.AxisListType
OP = mybir.AluOpType
ACTF = mybir.ActivationFunctionType

P = 128
HID = 256
HEADS = 8
DK = 32
SCALE = float(np.sqrt(DK))
CLIP = 10.0
CLIP_RAW = CLIP * SCALE  # clip before dividing by SCALE (equivalent)

N_CORES = 8

# dtype knobs ---------------------------------------------------------------
TABLE_DT = BF16   # dtype of kv_tab / q_tab in DRAM + gathered tiles
XF_DT = BF16      # dtype of the replicated x^T used for the kv projection
SEG_DT = BF16     # dtype of the segment-sum matmul operands (mask + wv)


def _cdiv(a, b):
    return -(-a // b)


def _np_dt(dt):
    return mybir.dt.np(dt)


class _Plan:
    """Host-side graph partition with load-balanced dst blocks."""

    def __init__(self, n_nodes, src, dst):
        self.n_nodes = n_nodes
        nblk_total = _cdiv(n_nodes, P)
        nblk_total = _cdiv(nblk_total, N_CORES) * N_CORES
        self.nblk = nblk_total // N_CORES          # blocks per core
        self.npad = self.nblk * P                  # node slots per core
        self.nkv = _cdiv(n_nodes, P) * P           # padded kv table rows
        self.split = (self.nkv // 2 // P) * P      # kv table half boundary

        deg = np.bincount(dst, minlength=n_nodes)
        # greedy balanced assignment: heaviest nodes first onto lightest block
        order = np.argsort(-deg, kind="stable")
        heap = [(0, b, 0) for b in range(nblk_total)]  # (load, block, n_nodes)
        heapq.heapify(heap)
        node_block = np.empty(n_nodes, np.int32)
        node_lane = np.empty(n_nodes, np.int32)
        for nid in order:
            load, b, cnt = heapq.heappop(heap)
            node_block[nid] = b
            node_lane[nid] = cnt
            cnt += 1
            if cnt < P:
                heapq.heappush(heap, (load + int(deg[nid]), b, cnt))
        self.node_block = node_block
        self.node_lane = node_lane
        # slot id within owner core: [0, npad)
        self.node_core = node_block // self.nblk
        self.node_slot = (node_block % self.nblk) * P + node_lane

        # per-(core,block,group) edge counts -> global S0/S1
        e_core = self.node_core[dst]
        e_blk = node_block[dst].astype(np.int64)
        e_grp = (src >= self.split).astype(np.int64)
        cnt = np.bincount(e_blk * 2 + e_grp, minlength=nblk_total * 2)
        cnt = cnt.reshape(nblk_total, 2)
        self.s0 = int(_cdiv(int(cnt[:, 0].max()), P))
        self.s1 = int(_cdiv(int(cnt[:, 1].max()), P))
        self.st = self.s0 + self.s1

        S0, S1, ST = self.s0, self.s1, self.st
        NBLK = self.nblk
        self.core_arrays = []
        for m in range(N_CORES):
            sel = e_core == m
            s_m = src[sel].astype(np.int64)
            blk = (e_blk[sel] % NBLK).astype(np.int64)
            dslot = self.node_slot[dst[sel]].astype(np.int64)
            grp = (s_m >= self.split).astype(np.int64)
            key = blk * 2 + grp
            order = np.argsort(key, kind="stable")
            s_m, blk, dslot, grp, key = (a[order] for a in
                                         (s_m, blk, dslot, grp, key))
            seg_cnt = np.bincount(key, minlength=NBLK * 2)
            start = np.zeros(NBLK * 2, np.int64)
            start[1:] = np.cumsum(seg_cnt)[:-1]
            j = np.arange(len(s_m)) - start[key]        # rank within segment
            i_blk = j + grp * (S0 * P)                  # slot id within block

            kv0 = np.zeros((NBLK, S0 * P), np.int64)
            kv1 = np.zeros((NBLK, S1 * P), np.int64)
            dstl = np.full((NBLK, ST * P), 999.0, np.float32)
            g0 = grp == 0
            kv0[blk[g0], j[g0]] = s_m[g0]
            g1 = grp == 1
            kv1[blk[g1], j[g1]] = s_m[g1] - self.split
            dstl[blk, i_blk] = (dslot % P).astype(np.float32)

            self.core_arrays.append({
                "kvi0": self._wrap16(kv0),
                "kvi1": self._wrap16(kv1),
                "dstl": self._slotf(dstl),
            })

    @staticmethod
    def _wrap16(x):
        """[NBLK, n] flat slot-order indices -> [128, NBLK*(n//16)] int16
        (index i at [i % 16, i // 16], replicated for the 8 Q7 cores)."""
        nblk, n = x.shape
        w = x.reshape(nblk, n // 16, 16).transpose(0, 2, 1)   # [NBLK, 16, n/16]
        w = np.tile(w, (1, 8, 1))                             # [NBLK, 128, n/16]
        w = w.transpose(1, 0, 2).reshape(P, nblk * (n // 16))
        return np.ascontiguousarray(w.astype(np.int16))

    @staticmethod
    def _slotf(x):
        """[NBLK, n] slot-order floats -> [128, NBLK*(n//128)] (p = slot%128)."""
        nblk, n = x.shape
        w = x.reshape(nblk, n // P, P).transpose(2, 0, 1)
        return np.ascontiguousarray(w.reshape(P, nblk * (n // P)))


def _build_program(plan):
    S0, S1, ST = plan.s0, plan.s1, plan.st
    NBLK = plan.nblk
    NPAD = plan.npad
    NKV = plan.nkv
    SPLIT = plan.split

    nc = bacc.Bacc("TRN2", target_bir_lowering=False, debug=False,
                   num_devices=N_CORES)

    def inp(name, shape, dt):
        return nc.dram_tensor(name, shape, dt, kind="ExternalInput").ap()

    xT_own = inp("xT_own", [2, P, NPAD], F32R)
    xT_full = inp("xT_full", [2, P, NKV], XF_DT)
    wqT = inp("wqT", [2, P, HID], F32R)
    wkvT = inp("wkvT", [2, P, 2 * HID], XF_DT)
    woT = inp("woT", [2, P, HID], F32)
    bq_rep = inp("bq_rep", [P, HID], F32)
    bo_rep = inp("bo_rep", [P, HID], F32)
    kvi0_in = inp("kvi0", [P, NBLK * S0 * 8], I16)
    kvi1_in = inp("kvi1", [P, NBLK * S1 * 8], I16)
    dstl_in = inp("dstl", [P, NBLK * ST], F32)
    iota_in = inp("iota_row", [P, P], F32)

    out_ap = nc.dram_tensor("out", [NPAD, HID], F32, kind="ExternalOutput").ap()

    with tile.TileContext(nc) as tc, ExitStack() as ctx:
        dram = ctx.enter_context(tc.tile_pool(name="dram", bufs=1, space="DRAM"))
        q_tab = dram.tile([NPAD, HID], TABLE_DT)
        kv_tab = dram.tile([NKV, 2 * HID], TABLE_DT)

        const = ctx.enter_context(tc.tile_pool(name="const", bufs=1))
        nc.gpsimd.load_library(library_config.mlp)
        ident = const.tile([P, P], F32)
        make_identity(nc, ident[:])
        ident_bf = const.tile([P, P], SEG_DT)
        make_identity(nc, ident_bf[:])
        iota_sb = const.tile([P, P], F32)
        nc.sync.dma_start(out=iota_sb[:], in_=iota_in[:])
        bq_sb = const.tile([P, HID], F32)
        nc.sync.dma_start(out=bq_sb[:], in_=bq_rep[:])
        bo_sb = const.tile([P, HID], F32)
        nc.sync.dma_start(out=bo_sb[:], in_=bo_rep[:])
        wq_sb = const.tile([P, 2, HID], F32R)
        wkv_sb = const.tile([P, 2, 2 * HID], XF_DT)
        wo_sb = const.tile([P, 2, HID], F32)
        for c in range(2):
            nc.sync.dma_start(out=wq_sb[:, c, :], in_=wqT[c])
            nc.sync.dma_start(out=wkv_sb[:, c, :], in_=wkvT[c])
            nc.sync.dma_start(out=wo_sb[:, c, :], in_=woT[c])
        kvi0_sb = const.tile([P, NBLK * S0 * 8], I16)
        nc.sync.dma_start(out=kvi0_sb[:], in_=kvi0_in[:])
        kvi1_sb = const.tile([P, NBLK * S1 * 8], I16)
        nc.sync.dma_start(out=kvi1_sb[:], in_=kvi1_in[:])
        dstl_sb = const.tile([P, NBLK * ST], F32)
        nc.sync.dma_start(out=dstl_sb[:], in_=dstl_in[:])

        # ---------------- phase 1: projections -> q_tab, kv_tab ------------
        with ExitStack() as p1:
            own_pool = p1.enter_context(tc.tile_pool(name="own", bufs=1))
            slab_pool = p1.enter_context(tc.tile_pool(name="slab", bufs=2))
            qs_pool = p1.enter_context(tc.tile_pool(name="qs", bufs=3))
            kvs_pool = p1.enter_context(tc.tile_pool(name="kvs", bufs=4))
            psq = p1.enter_context(tc.tile_pool(name="psq", bufs=2, space="PSUM"))
            pskv = p1.enter_context(tc.tile_pool(name="pskv", bufs=3, space="PSUM"))

            xo_sb = own_pool.tile([P, 2, NPAD], F32R)
            for c in range(2):
                nc.sync.dma_start(out=xo_sb[:, c, :], in_=xT_own[c])

            for b in range(NBLK):
                ps = psq.tile([P, HID], F32, space="PSUM")
                for c in range(2):
                    nc.tensor.matmul(
                        out=ps[:],
                        lhsT=xo_sb[:, c, b * P:(b + 1) * P],
                        rhs=wq_sb[:, c, :],
                        start=(c == 0), stop=(c == 1))
                qs = qs_pool.tile([P, HID], TABLE_DT)
                nc.vector.tensor_tensor(qs[:], ps[:], bq_sb[:], op=OP.add)
                nc.sync.dma_start(out=q_tab[b * P:(b + 1) * P, :], in_=qs[:])

            SLAB = 2048
            nslab = _cdiv(NKV, SLAB)
            for s in range(nslab):
                w = min(SLAB, NKV - s * SLAB)
                xs = slab_pool.tile([P, 2, SLAB], XF_DT)
                for c in range(2):
                    nc.sync.dma_start(out=xs[:, c, :w],
                                      in_=xT_full[c, :, s * SLAB:s * SLAB + w])
                for k in range(w // P):
                    ps = pskv.tile([P, 2 * HID], F32, space="PSUM")
                    for c in range(2):
                        nc.tensor.matmul(out=ps[:],
                                         lhsT=xs[:, c, k * P:(k + 1) * P],
                                         rhs=wkv_sb[:, c, :],
                                         start=(c == 0), stop=(c == 1))
                    kvs = kvs_pool.tile([P, 2 * HID], TABLE_DT)
                    row = s * SLAB // P + k
                    if row % 4 == 0:
                        nc.vector.tensor_copy(kvs[:], ps[:])
                    else:
                        nc.scalar.copy(kvs[:], ps[:])
                    nc.sync.dma_start(out=kv_tab[row * P:(row + 1) * P, :],
                                      in_=kvs[:])

        # ---------------- phase 2+3: edge pipeline per dst block -----------
        kv_pool = ctx.enter_context(tc.tile_pool(name="kvt", bufs=2))
        qb_pool = ctx.enter_context(tc.tile_pool(name="qb", bufs=2))
        me_pool = ctx.enter_context(tc.tile_pool(name="me", bufs=2))
        mp_pool = ctx.enter_context(tc.tile_pool(name="mp", bufs=3))
        qd_pool = ctx.enter_context(tc.tile_pool(name="qds", bufs=3))
        prod_pool = ctx.enter_context(tc.tile_pool(name="prod", bufs=2))
        work_pool = ctx.enter_context(tc.tile_pool(name="work", bufs=2))
        sc_pool = ctx.enter_context(tc.tile_pool(name="sc", bufs=2))
        se_pool = ctx.enter_context(tc.tile_pool(name="se", bufs=2))
        rz_pool = ctx.enter_context(tc.tile_pool(name="rz", bufs=2))
        op_pool = ctx.enter_context(tc.tile_pool(name="opre", bufs=2))
        ots_pool = ctx.enter_context(tc.tile_pool(name="ots", bufs=2))
        outs_pool = ctx.enter_context(tc.tile_pool(name="outs", bufs=3))
        acc_ps = ctx.enter_context(tc.tile_pool(name="acc", bufs=2, space="PSUM"))
        mp_psp = ctx.enter_context(tc.tile_pool(name="mpp", bufs=2, space="PSUM"))
        qd_psp = ctx.enter_context(tc.tile_pool(name="qdp", bufs=2, space="PSUM"))
        ot_psp = ctx.enter_context(tc.tile_pool(name="otp", bufs=1, space="PSUM"))
        out_psp = ctx.enter_context(tc.tile_pool(name="outp", bufs=1, space="PSUM"))

        MAXSUB = 8  # dma_gather handles at most 1024 indices per call

        def gather_chunks(out_tile, t_lo, n_sub, in_ap, idx_sb, col_base, elem):
            off = 0
            while off < n_sub:
                c = min(MAXSUB, n_sub - off)
                nc.gpsimd.dma_gather(
                    out_ap=out_tile[:, t_lo + off:t_lo + off + c, :],
                    in_ap=in_ap,
                    idxs_ap=idx_sb[:, col_base + off * 8:col_base + (off + c) * 8],
                    num_idxs=c * P, num_idxs_reg=c * P, elem_size=elem)
                off += c

        CEX = 2  # expansion chunk (subtiles per PSUM qd tile)

        for b in range(NBLK):
            kvt = kv_pool.tile([P, ST, 2 * HID], TABLE_DT)
            gather_chunks(kvt, 0, S0, kv_tab[0:SPLIT, :], kvi0_sb,
                          b * S0 * 8, 2 * HID)
            gather_chunks(kvt, S0, S1, kv_tab[SPLIT:, :], kvi1_sb,
                          b * S1 * 8, 2 * HID)
            qb = qb_pool.tile([P, HID], TABLE_DT)
            nc.sync.dma_start(out=qb[:], in_=q_tab[b * P:(b + 1) * P, :])

            me = me_pool.tile([P, ST, P], SEG_DT)
            nc.vector.tensor_tensor(
                me[:],
                dstl_sb[:, b * ST:(b + 1) * ST].unsqueeze(2)
                .broadcast_to([P, ST, P]),
                iota_sb[:].unsqueeze(1).broadcast_to([P, ST, P]),
                op=OP.is_equal)

            # expand q[dst] per edge on PE: M' = me^T, q_dst = M'^T.T @ q_B
            prod = prod_pool.tile([P, ST, HID], SEG_DT)
            for lo in range(0, ST, CEX):
                c = min(CEX, ST - lo)
                mp_ps = mp_psp.tile([P, CEX, P], SEG_DT, space="PSUM")
                for i in range(c):
                    nc.tensor.transpose(mp_ps[:, i, :], me[:, lo + i, :],
                                        ident_bf[:])
                mp_sb = mp_pool.tile([P, CEX, P], SEG_DT)
                nc.scalar.copy(mp_sb[:, :c, :], mp_ps[:, :c, :])
                qd_ps = qd_psp.tile([P, CEX, HID], F32, space="PSUM")
                for i in range(c):
                    nc.tensor.matmul(out=qd_ps[:, i, :],
                                     lhsT=mp_sb[:, i, :], rhs=qb[:],
                                     start=True, stop=True)
                qd_sb = qd_pool.tile([P, CEX, HID], TABLE_DT)
                nc.scalar.copy(qd_sb[:, :c, :], qd_ps[:, :c, :])
                nc.vector.tensor_tensor(prod[:, lo:lo + c, :],
                                        kvt[:, lo:lo + c, 0:HID],
                                        qd_sb[:, :c, :], op=OP.mult)
            sc = sc_pool.tile([P, ST, HEADS], F32)
            nc.vector.tensor_reduce(
                sc[:],
                prod[:].rearrange("p s (h d) -> p s h d", h=HEADS),
                axis=AX.X, op=OP.add)
            nc.vector.tensor_scalar(sc[:], sc[:], CLIP_RAW, -CLIP_RAW,
                                    op0=OP.min, op1=OP.max)
            se = se_pool.tile([P, ST, HEADS], F32)
            nc.scalar.activation(se[:], sc[:], func=ACTF.Exp, scale=1.0 / SCALE)

            work = work_pool.tile([P, ST, HID + HEADS], SEG_DT)
            nc.vector.tensor_tensor(
                work[:, :, 0:HID].rearrange("p s (h d) -> p s h d", h=HEADS),
                kvt[:, :, HID:2 * HID].rearrange("p s (h d) -> p s h d", h=HEADS),
                se[:].unsqueeze(3).broadcast_to([P, ST, HEADS, DK]),
                op=OP.mult)
            nc.vector.tensor_copy(work[:, :, HID:HID + HEADS], se[:])

            acc = acc_ps.tile([P, HID + HEADS], F32, space="PSUM")
            for t in range(ST):
                nc.tensor.matmul(out=acc[:],
                                 lhsT=me[:, t, :],
                                 rhs=work[:, t, 0:HID + HEADS],
                                 start=(t == 0), stop=(t == ST - 1))

            # normalize + output projection
            nc.vector.tensor_scalar_add(acc[:, HID:HID + HEADS],
                                        acc[:, HID:HID + HEADS], 1e-30)
            rz = rz_pool.tile([P, HEADS], F32)
            nc.vector.reciprocal(rz[:], acc[:, HID:HID + HEADS])
            op_sb = op_pool.tile([P, HID], F32)
            nc.vector.tensor_tensor(
                op_sb[:].rearrange("p (h d) -> p h d", h=HEADS),
                acc[:, 0:HID].rearrange("p (h d) -> p h d", h=HEADS),
                rz[:].unsqueeze(2).broadcast_to([P, HEADS, DK]),
                op=OP.mult)
            ot_ps = ot_psp.tile([P, 2, P], F32, space="PSUM")
            for c in range(2):
                nc.tensor.transpose(ot_ps[:, c, :], op_sb[:, c * P:(c + 1) * P],
                                    ident[:])
            ot_sb = ots_pool.tile([P, 2, P], F32)
            nc.scalar.copy(ot_sb[:], ot_ps[:])
            out_ps = out_psp.tile([P, HID], F32, space="PSUM")
            for c in range(2):
                nc.tensor.matmul(out=out_ps[:],
                                 lhsT=ot_sb[:, c, :],
                                 rhs=wo_sb[:, c, :],
                                 start=(c == 0), stop=(c == 1))
            out_sb = outs_pool.tile([P, HID], F32)
            nc.vector.tensor_tensor(out_sb[:], out_ps[:], bo_sb[:], op=OP.add)
            nc.sync.dma_start(out=out_ap[b * P:(b + 1) * P, :], in_=out_sb[:])

    nc.compile()
    return nc


_PROG_CACHE = {}


def _get_program(plan):
    key = (plan.n_nodes, plan.s0, plan.s1)
    if key not in _PROG_CACHE:
        _PROG_CACHE[key] = _build_program(plan)
    return _PROG_CACHE[key]


def prepare(inputs, Wq, bq, Wk, Wv, Wo, bo, src, dst):
    inputs = np.asarray(inputs, np.float32)
    Wq = np.asarray(Wq, np.float32)
    bq = np.asarray(bq, np.float32)
    Wk = np.asarray(Wk, np.float32)
    Wv = np.asarray(Wv, np.float32)
    Wo = np.asarray(Wo, np.float32)
    bo = np.asarray(bo, np.float32)
    src = np.asarray(src, np.int64)
    dst = np.asarray(dst, np.int64)

    n, hid = inputs.shape
    assert hid == HID
    plan = _Plan(n, src, dst)
    nc = _get_program(plan)

    xT_full = np.zeros((2, P, plan.nkv), np.float32)
    xT_full[0, :, :n] = inputs.T[0:P, :]
    xT_full[1, :, :n] = inputs.T[P:2 * P, :]
    xT_full = xT_full.astype(_np_dt(XF_DT))
    wqT = np.ascontiguousarray(Wq.T.reshape(2, P, HID))
    wkvT = np.concatenate([Wk.T, Wv.T], axis=1).reshape(2, P, 2 * HID)
    wkvT = np.ascontiguousarray(wkvT).astype(_np_dt(XF_DT))
    woT = np.ascontiguousarray(Wo.T.reshape(2, P, HID))
    bq_rep = np.ascontiguousarray(np.broadcast_to(bq, (P, HID)))
    bo_rep = np.ascontiguousarray(np.broadcast_to(bo, (P, HID)))
    iota_row = np.ascontiguousarray(
        np.broadcast_to(np.arange(P, dtype=np.float32), (P, P)))

    # per-core x_own in (block, lane) slot order
    in_maps = []
    for m in range(N_CORES):
        sel = plan.node_core == m
        nids = np.nonzero(sel)[0]
        slots = plan.node_slot[nids]
        xo_rows = np.zeros((plan.npad, HID), np.float32)
        xo_rows[slots] = inputs[nids]
        xo = np.ascontiguousarray(
            xo_rows.T.reshape(2, P, plan.npad))
        ca = plan.core_arrays[m]
        in_maps.append({
            "xT_own": xo,
            "xT_full": xT_full,
            "wqT": wqT,
            "wkvT": wkvT,
            "woT": woT,
            "bq_rep": bq_rep,
            "bo_rep": bo_rep,
            "kvi0": ca["kvi0"],
            "kvi1": ca["kvi1"],
            "dstl": ca["dstl"],
            "iota_row": iota_row,
        })
    return plan, nc, in_maps


def assemble(plan, res):
    n = plan.n_nodes
    out = np.zeros((n, HID), np.float32)
    for m in range(N_CORES):
        sel = plan.node_core == m
        nids = np.nonzero(sel)[0]
        slots = plan.node_slot[nids]
        out[nids] = np.asarray(res.results[m]["out"], np.float32)[slots]
    return out


def kernel(**inputs):
    plan, nc, in_maps = prepare(**inputs)
    res = run_bass_kernel_spmd(nc, in_maps, core_ids=list(range(N_CORES)))
    return assemble(plan, res)
